# revision 5
# baseline (speedup 1.0000x reference)
"""ConvChunk2d patch-extraction kernel for Trainium2 (8 NeuronCores).

Reference computes, for x of shape (8, 64, 128, 128):
    out[n, y*128 + xx, c, a, b] = xpad[n, (a*192 + b*64 + c) // 9, y + a, xx + b]
with xpad zero-padded by 1 on H/W, output shape (8*16384, 64, 3, 3).

Pure data movement (gather + replication), memory-bound.  Strategy:
data-parallel over batch (1 image per core), with the dominant HBM
traffic (the 9x-replicated output) in bf16: harness tolerance 2e-2 >>
bf16's 2^-8 max relative rounding error, and bf16 keeps f32's exponent
range so tiny values stay accurate.

Key observation: the source channel is ch = (64*p + c) // 9 with
p = 3a + b, so the kernel row-shift a only ever reads channels
  a=0 -> ch in [0, 21],  a=1 -> ch in [21, 42],  a=2 -> ch in [42, 63].
The host pre-builds A[blk, y, ch+a, xcol] = xpad[ch, y - 1 + a, x0+xcol]
(66 rows instead of 3*64): zero padding, partition row-shifts and
x-block windows all baked in, so each block needs exactly one
full-128-partition 2D DMA (the only kind that spreads across all 16
SDMA engines) and the device reads only ~1.03x the input.

Measured engine facts (clean microbench, this container):
  - strided scatter with f32 dst: ACT ~0.36 ns/el (fastest by 4x),
    DVE ~1.6 ns/el, GPSIMD ~1.9 ns/el at e=256
  - any *bf16-dst strided* write is 3-4x slower on every engine
    (DVE's 2x 16-bit mode needs unit stride + 4B alignment), so the
    scatter goes to an f32 tile and a separate *contiguous* f32->bf16
    convert follows: DVE 0.53 ns/el, ACT 0.85, GPSIMD 3.3.
Per x-block: 81 strided copies A -> T32[y, xx, c, p] (for fixed p and
s = (64p+c) mod 9 the columns j = c*9 + p are one affine family over
ch), then contiguous convert chunks T32 -> T16, then one 2D store of
T16's [128, xb*576].  Jobs spread greedily across engines by measured
cost; block widths ramp [8, 24, 32, ...] so the store stream starts
early.  Loads go on the Activation DMA queue so they never queue
behind a store that is still waiting on its convert.
"""

import math

import numpy as np
import ml_dtypes

import concourse.bacc as bacc
import concourse.bass as bass
import concourse.mybir as mybir
from concourse.bass_utils import run_bass_kernel_spmd
from concourse.tile import TileContext

N, C, H, W = 8, 64, 128, 128
K = 3
L = H * W
J = C * K * K  # 576 output columns per spatial location
BLOCKS = [8, 24, 32, 32, 32]  # x-block widths (sum = W); ramped start
XMAX = 32
XAP = XMAX + 2  # A tile row width (padded; every block uses cols 0:xb+2)
NB = len(BLOCKS)
CH3 = 3 * 22  # channel rows kept per shift a: ch+a for ch in a's range
NCVT = 6  # convert chunks per block
F32 = mybir.dt.float32
BF16 = mybir.dt.bfloat16
NPBF16 = ml_dtypes.bfloat16


def _jobs():
    """(a, b, ch_lo, cnt, c0, p) for each affine copy family."""
    jobs = []
    for p in range(K * K):
        a, b = divmod(p, K)
        for s in range(9):
            ch_lo = math.ceil((64 * p - s) / 9)
            ch_hi = (63 + 64 * p - s) // 9
            cnt = ch_hi - ch_lo + 1
            c0 = 9 * ch_lo + s - 64 * p
            jobs.append((a, b, ch_lo, cnt, c0, p))
    return jobs


# Measured per-op cost models (ns), clean: [DVE, ACT, GPSIMD]
def _scatter_cost(e):
    return (195 + 0.85 * e, 60 + 0.25 * e, 240 + 1.0 * e)


def _cvt_cost(e):
    return (85 + 0.53 * e, 100 + 0.85 * e, 400 + 3.3 * e)


def build_nc():
    nc = bacc.Bacc("TRN2")
    xh = nc.declare_dram_parameter("xh", [NB, 128, CH3, XAP], F32, isOutput=False)
    out = nc.declare_dram_parameter("out", [L, J], BF16, isOutput=True)

    with TileContext(nc) as tc:
        with (
            tc.tile_pool(name="a", bufs=2) as apool,
            tc.tile_pool(name="t32", bufs=1) as t32pool,
            tc.tile_pool(name="t16", bufs=2) as t16pool,
        ):
            jobs = _jobs()
            outr = out[:, :].rearrange("(y xx) j -> y xx j", xx=W)
            load = [0.0, 0.0, 0.0]

            def run(eng, dst, src):
                if eng == 0:
                    nc.vector.tensor_copy(dst, src)
                elif eng == 1:
                    nc.scalar.copy(dst, src)
                else:
                    nc.gpsimd.tensor_copy(dst, src)

            x0 = 0
            for blk, xb in enumerate(BLOCKS):
                A = apool.tile([128, CH3, XAP], F32, tag="a")
                nc.scalar.dma_start(out=A[:, :, :], in_=xh[blk, :, :, :])
                T32 = t32pool.tile([128, XMAX, C, K * K], F32, tag="t32")
                for a, b, ch_lo, cnt, c0, p in jobs:
                    dst = T32[
                        :, 0:xb, c0 : c0 + 9 * (cnt - 1) + 1 : 9, p
                    ].transpose([0, 2, 1])
                    src = A[:, ch_lo + a : ch_lo + a + cnt, b : b + xb]
                    import os
                    mode = os.environ.get("ASSIGN", "greedy")
                    if mode == "act_dve":
                        eng = 1
                    else:
                        costs = _scatter_cost(cnt * xb)
                        eng = min(range(3), key=lambda i: load[i] + costs[i])
                        load[eng] += costs[eng]
                    run(eng, dst, src)
                T16 = t16pool.tile([128, XMAX, C, K * K], BF16, tag="t16")
                fe = xb * J
                s32 = T32[:, 0:xb, :, :].rearrange("pp xx c q -> pp (xx c q)")
                s16 = T16[:, 0:xb, :, :].rearrange("pp xx c q -> pp (xx c q)")
                step = fe // NCVT // 32 * 32
                cuts = [i * step for i in range(NCVT)] + [fe]
                for k in range(NCVT):
                    e = cuts[k + 1] - cuts[k]
                    import os
                    mode = os.environ.get("ASSIGN", "greedy")
                    if mode == "act_dve":
                        eng = 0
                    else:
                        costs = _cvt_cost(e)
                        eng = min(range(3), key=lambda i: load[i] + costs[i])
                        load[eng] += costs[eng]
                    run(eng, s16[:, cuts[k] : cuts[k + 1]], s32[:, cuts[k] : cuts[k + 1]])
                nc.sync.dma_start(out=outr[:, x0 : x0 + xb, :], in_=s16)
                x0 += xb
    nc.finalize()
    return nc


def _prep(x):
    """(N, C, H, W) f32 -> per-core A[blk, y, ch+a, xcol] layouts."""
    xp = np.zeros((N, C, H + 2, W + 2), np.float32)
    xp[:, :, 1 : H + 1, 1 : W + 1] = x
    A = np.zeros((N, NB, 128, CH3, XAP), np.float32)
    x0 = 0
    for blk, xb in enumerate(BLOCKS):
        for a in range(3):
            ch0 = 21 * a
            A[:, blk, :, 22 * a : 22 * (a + 1), 0 : xb + 2] = xp[
                :, ch0 : ch0 + 22, a : a + H, x0 : x0 + xb + 2
            ].transpose(0, 2, 1, 3)
        x0 += xb
    return A


def _run(x, **kw):
    x = np.ascontiguousarray(np.asarray(x, dtype=np.float32))
    assert x.shape == (N, C, H, W), x.shape
    xh = _prep(x)
    nc = build_nc()
    in_maps = [{"xh": xh[n]} for n in range(N)]
    res = run_bass_kernel_spmd(nc, in_maps, list(range(N)), **kw)
    outs = [
        np.asarray(res.results[i]["out"])
        .astype(np.float32)
        .reshape(L, C, K, K)
        for i in range(N)
    ]
    return np.concatenate(outs, axis=0), res


def kernel(x):
    return _run(x)[0]


# revision 6
# speedup vs baseline: 1.5344x; 1.5344x over previous
"""ConvChunk2d patch-extraction kernel for Trainium2 (8 NeuronCores).

Reference computes, for x of shape (8, 64, 128, 128):
    out[n, y*128 + xx, c, a, b] = xpad[n, (a*192 + b*64 + c) // 9, y + a, xx + b]
with xpad zero-padded by 1 on H/W, output shape (8*16384, 64, 3, 3).

Pure data movement (gather + replication), memory-bound.  Strategy:
data-parallel over batch (1 image per core), with the dominant HBM
traffic (the 9x-replicated output) in bf16: harness tolerance 2e-2 >>
bf16's 2^-8 max relative rounding error, and bf16 keeps f32's exponent
range so tiny values stay accurate.

Key observation: the source channel is ch = (64*p + c) // 9 with
p = 3a + b, so the kernel row-shift a only ever reads channels
  a=0 -> ch in [0, 21],  a=1 -> ch in [21, 42],  a=2 -> ch in [42, 63].
The host pre-builds A[blk, y, ch+a, xcol] = xpad[ch, y - 1 + a, x0+xcol]
(66 rows instead of 3*64): zero padding, partition row-shifts and
x-block windows all baked in, so each block needs exactly one
full-128-partition 2D DMA (the only kind that spreads across all 16
SDMA engines) and the device reads only ~1.03x the input.

Measured engine facts (clean microbench, this container):
  - strided scatter with f32 dst: ACT ~0.36 ns/el (fastest by 4x),
    DVE ~1.6 ns/el, GPSIMD ~1.9 ns/el at e=256
  - any *bf16-dst strided* write is 3-4x slower on every engine
    (DVE's 2x 16-bit mode needs unit stride + 4B alignment), so the
    scatter goes to an f32 tile and a separate *contiguous* f32->bf16
    convert follows: DVE 0.53 ns/el, ACT 0.85, GPSIMD 3.3.
Per x-block: 81 strided copies A -> T32[y, xx, c, p] (for fixed p and
s = (64p+c) mod 9 the columns j = c*9 + p are one affine family over
ch), then contiguous convert chunks T32 -> T16, then one 2D store of
T16's [128, xb*576].  Jobs spread greedily across engines by measured
cost; block widths ramp [8, 24, 32, ...] so the store stream starts
early.  Loads go on the Activation DMA queue so they never queue
behind a store that is still waiting on its convert.
"""

import math

import numpy as np
import ml_dtypes

import concourse.bacc as bacc
import concourse.bass as bass
import concourse.mybir as mybir
from concourse.bass_utils import run_bass_kernel_spmd
from concourse.tile import TileContext

N, C, H, W = 8, 64, 128, 128
K = 3
L = H * W
J = C * K * K  # 576 output columns per spatial location
BLOCKS = [8, 24, 32, 32, 32]  # x-block widths (sum = W); ramped start
XMAX = 32
XAP = XMAX + 2  # A tile row width (padded; every block uses cols 0:xb+2)
NB = len(BLOCKS)
CH3 = 3 * 22  # channel rows kept per shift a: ch+a for ch in a's range
NCVT = 6  # convert chunks per block
F32 = mybir.dt.float32
BF16 = mybir.dt.bfloat16
NPBF16 = ml_dtypes.bfloat16


def _jobs():
    """(a, b, ch_lo, cnt, c0, p) for each affine copy family."""
    jobs = []
    for p in range(K * K):
        a, b = divmod(p, K)
        for s in range(9):
            ch_lo = math.ceil((64 * p - s) / 9)
            ch_hi = (63 + 64 * p - s) // 9
            cnt = ch_hi - ch_lo + 1
            c0 = 9 * ch_lo + s - 64 * p
            jobs.append((a, b, ch_lo, cnt, c0, p))
    return jobs


# Measured per-op cost models (ns), clean: [DVE, ACT, GPSIMD]
def _scatter_cost(e):
    return (200 + 2.8 * e, 100 + 1.7 * e, 240 + 3.0 * e)


def _cvt_cost(e):
    return (100 + 0.45 * e, 120 + 0.9 * e, 400 + 4.5 * e)


def build_nc():
    nc = bacc.Bacc("TRN2")
    xh = nc.declare_dram_parameter("xh", [NB, 128, CH3, XAP], F32, isOutput=False)
    out = nc.declare_dram_parameter("out", [L, J], BF16, isOutput=True)

    with TileContext(nc) as tc:
        with (
            tc.tile_pool(name="a", bufs=2) as apool,
            tc.tile_pool(name="t32", bufs=1) as t32pool,
            tc.tile_pool(name="t16", bufs=2) as t16pool,
        ):
            jobs = _jobs()
            outr = out[:, :].rearrange("(y xx) j -> y xx j", xx=W)
            load = [0.0, 0.0, 0.0]

            def run(eng, dst, src):
                if eng == 0:
                    nc.vector.tensor_copy(dst, src)
                elif eng == 1:
                    nc.scalar.copy(dst, src)
                else:
                    nc.gpsimd.tensor_copy(dst, src)

            x0 = 0
            for blk, xb in enumerate(BLOCKS):
                A = apool.tile([128, CH3, XAP], F32, tag="a")
                nc.sync.dma_start(out=A[:, :, :], in_=xh[blk, :, :, :])
                T32 = t32pool.tile([128, XMAX, C, K * K], F32, tag="t32")
                for a, b, ch_lo, cnt, c0, p in jobs:
                    dst = T32[
                        :, 0:xb, c0 : c0 + 9 * (cnt - 1) + 1 : 9, p
                    ].transpose([0, 2, 1])
                    src = A[:, ch_lo + a : ch_lo + a + cnt, b : b + xb]
                    import os
                    mode = os.environ.get("ASSIGN", "greedy")
                    if mode == "act_dve":
                        eng = 1
                    else:
                        costs = _scatter_cost(cnt * xb)
                        eng = min(range(3), key=lambda i: load[i] + costs[i])
                        load[eng] += costs[eng]
                    run(eng, dst, src)
                T16 = t16pool.tile([128, XMAX, C, K * K], BF16, tag="t16")
                fe = xb * J
                s32 = T32[:, 0:xb, :, :].rearrange("pp xx c q -> pp (xx c q)")
                s16 = T16[:, 0:xb, :, :].rearrange("pp xx c q -> pp (xx c q)")
                step = fe // NCVT // 32 * 32
                cuts = [i * step for i in range(NCVT)] + [fe]
                for k in range(NCVT):
                    e = cuts[k + 1] - cuts[k]
                    import os
                    mode = os.environ.get("ASSIGN", "greedy")
                    if mode == "act_dve":
                        eng = 0
                    else:
                        costs = _cvt_cost(e)
                        eng = min(range(3), key=lambda i: load[i] + costs[i])
                        load[eng] += costs[eng]
                    run(eng, s16[:, cuts[k] : cuts[k + 1]], s32[:, cuts[k] : cuts[k + 1]])
                nc.sync.dma_start(out=outr[:, x0 : x0 + xb, :], in_=s16)
                x0 += xb
    nc.finalize()
    return nc


def _prep(x):
    """(N, C, H, W) f32 -> per-core A[blk, y, ch+a, xcol] layouts."""
    xp = np.zeros((N, C, H + 2, W + 2), np.float32)
    xp[:, :, 1 : H + 1, 1 : W + 1] = x
    A = np.zeros((N, NB, 128, CH3, XAP), np.float32)
    x0 = 0
    for blk, xb in enumerate(BLOCKS):
        for a in range(3):
            ch0 = 21 * a
            A[:, blk, :, 22 * a : 22 * (a + 1), 0 : xb + 2] = xp[
                :, ch0 : ch0 + 22, a : a + H, x0 : x0 + xb + 2
            ].transpose(0, 2, 1, 3)
        x0 += xb
    return A


def _run(x, **kw):
    x = np.ascontiguousarray(np.asarray(x, dtype=np.float32))
    assert x.shape == (N, C, H, W), x.shape
    xh = _prep(x)
    nc = build_nc()
    in_maps = [{"xh": xh[n]} for n in range(N)]
    res = run_bass_kernel_spmd(nc, in_maps, list(range(N)), **kw)
    outs = [
        np.asarray(res.results[i]["out"])
        .astype(np.float32)
        .reshape(L, C, K, K)
        for i in range(N)
    ]
    return np.concatenate(outs, axis=0), res


def kernel(x):
    return _run(x)[0]
